# revision 1
# baseline (speedup 1.0000x reference)
"""Trainium2 Bass kernel for nn_BilinearChebConv (bilinear Chebyshev graph conv).

out[o] = sum_{i,j} theta[i,j,0,o] * T_i(Lr) @ x @ T_j(Lc) + bias[o]

Restructured to avoid materializing the Chebyshev bases:
  per core c (rows m_c = c*192 .. c*192+191):
    B_i = T_i(Lr)[:, m_c]            (thin column-slice Chebyshev recursion)
    W_0 = x^T @ [B_0 .. B_4]         (n on partitions, (i, m) stacked on free)
    W_j = 2 Lc W_{j-1} - W_{j-2}     (all 5 i-chains in one recursion)
    out[o, m_c, n] = sum_ij theta[ijo] W_j[n, (i, m)] + bias[o]

All heavy matmuls run as float32r (full PE rate at free-dim >= 256). Both Lr
and Lc are symmetric, so left-multiplication uses them directly as lhsT.
The theta contraction needs (i, j) on partitions; W tiles are spilled to DRAM
and gathered back as Zf[25, (n, m)] tiles (partition-collapsing SBUF->SBUF
DMAs are pathologically slow, the DRAM round-trip is not).
M (rows) is sharded across the 8 cores; per-core column slices are prepared
host-side so a single core-agnostic NEFF runs SPMD on all 8 cores.
"""

import sys

sys.path.insert(0, "/opt/trn_rl_repo")

import numpy as np
import ml_dtypes

M = 1536
N = 1536
NCORES = 8
ML = M // NCORES          # 192 rows per core
MT = 64                   # m-third width (192 = 3 * 64)
NTH = ML // MT            # 3 thirds
OUT = 32
KB = M // 128             # 12 partition blocks
P = 128
BP = 256                  # padded per-i column block in B (192 real + 64 zero)

_BUILT = None


def _build_program():
    import concourse.bacc as bacc
    import concourse.mybir as mybir
    from concourse import tile

    F32R = mybir.dt.float32r
    F32 = mybir.dt.float32
    BF16 = mybir.dt.bfloat16

    nc = bacc.Bacc(num_devices=NCORES)

    x_d = nc.dram_tensor("x", [M, N], F32R, kind="ExternalInput")
    lr2_d = nc.dram_tensor("lr2", [M, M], F32R, kind="ExternalInput")
    lc2_d = nc.dram_tensor("lc2", [N, N], F32R, kind="ExternalInput")
    # b01: cols 0..191 = E_c (identity slice), 192..255 zero,
    #      cols 256..447 = Lr[:, m_c],         448..511 zero
    b01_d = nc.dram_tensor("b01", [M, 2 * BP], F32R, kind="ExternalInput")
    negi_d = nc.dram_tensor("negi", [P, P], F32R, kind="ExternalInput")
    thf_d = nc.dram_tensor("thetaf", [25, OUT], BF16, kind="ExternalInput")
    bias_d = nc.dram_tensor("biasr", [P, 1], F32, kind="ExternalInput")
    w0_d = nc.dram_tensor("w0scratch", [NTH, 5, KB, P, MT], F32R, kind="Internal")
    w_d = nc.dram_tensor("wscratch", [NTH, 4, 5, KB, P, MT], BF16, kind="Internal")
    w0b_d = nc.dram_tensor("w0bscratch", [NTH, 5, KB, P, MT], BF16, kind="Internal")
    out_d = nc.dram_tensor("outc", [OUT, ML, N], F32, kind="ExternalOutput")

    with tile.TileContext(nc) as tc:
        with tc.tile_pool(name="const", bufs=1) as constp:
            negit = constp.tile([P, P], F32R, tag="negi")
            nc.sync.dma_start(negit[:], negi_d[:])
            thsb = constp.tile([25, OUT], BF16, tag="thf")
            nc.sync.dma_start(thsb[:], thf_d[:])
            biast = constp.tile([P, 1], F32, tag="bias")
            nc.sync.dma_start(biast[:], bias_d[:])

            # ---------------- Phase R: row stage ----------------
            with (
                tc.tile_pool(name="lr2p", bufs=1) as lr2p,
                tc.tile_pool(name="bpad", bufs=1) as bpadp,
                tc.tile_pool(name="xs", bufs=6) as xp,
                tc.tile_pool(name="w0sb", bufs=2) as w0p,
                tc.tile_pool(name="brps", bufs=2, space="PSUM") as brps,
                tc.tile_pool(name="w0ps", bufs=2, space="PSUM") as w0ps,
            ):
                lr2t = []
                for k in range(KB):
                    t_ = lr2p.tile([P, M], F32R, tag=f"lr{k}", name=f"lr2t{k}")
                    nc.sync.dma_start(t_[:], lr2_d[k * P : (k + 1) * P, :])
                    lr2t.append(t_)
                bt = []
                for k in range(KB):
                    t_ = bpadp.tile([P, 5 * BP], F32R, tag=f"bp{k}", name=f"bt{k}")
                    nc.sync.dma_start(t_[:, 0 : 2 * BP], b01_d[k * P : (k + 1) * P, :])
                    bt.append(t_)

                # B recursion: B_i = Lr2 @ B_{i-1} - B_{i-2}, i = 2..4
                for i in range(2, 5):
                    for p in range(KB):
                        ps = brps.tile([P, BP], F32, tag="brec", name="brps")
                        for k in range(KB):
                            nc.tensor.matmul(
                                ps[:],
                                lhsT=lr2t[k][:, p * P : (p + 1) * P],
                                rhs=bt[k][:, (i - 1) * BP : i * BP],
                                start=(k == 0),
                                stop=False,
                            )
                        nc.tensor.matmul(
                            ps[:],
                            lhsT=negit[:],
                            rhs=bt[p][:, (i - 2) * BP : (i - 1) * BP],
                            start=False,
                            stop=True,
                        )
                        nc.vector.tensor_copy(bt[p][:, i * BP : (i + 1) * BP], ps[:])

                # W0 = x^T @ B  (keep real 192-wide column blocks only)
                segs = [(0, 512), (512, 512), (1024, 256)]
                for nb in range(KB):
                    pss = [
                        w0ps.tile([P, sz], F32, tag=f"w0s{si}", name=f"w0ps{si}")
                        for si, (_, sz) in enumerate(segs)
                    ]
                    for k in range(KB):
                        xt = xp.tile([P, P], F32R, tag="x", name="xt")
                        nc.gpsimd.dma_start(
                            xt[:], x_d[k * P : (k + 1) * P, nb * P : (nb + 1) * P]
                        )
                        for ps, (off, sz) in zip(pss, segs):
                            nc.tensor.matmul(
                                ps[:],
                                lhsT=xt[:],
                                rhs=bt[k][:, off : off + sz],
                                start=(k == 0),
                                stop=(k == KB - 1),
                            )
                    w0sb = w0p.tile([P, 5 * ML], F32R, tag="w0", name="w0sb")
                    for i in range(5):
                        g = i * BP
                        si = g // 512
                        loc = g - si * 512
                        nc.vector.tensor_copy(
                            w0sb[:, i * ML : i * ML + ML], pss[si][:, loc : loc + ML]
                        )
                    w0sb16 = w0p.tile([P, 5 * ML], BF16, tag="w0b", name="w0sb16")
                    nc.vector.tensor_copy(w0sb16[:], w0sb[:])
                    w0v = w0sb.rearrange("n (i m) -> n i m", i=5)
                    w0v16 = w0sb16.rearrange("n (i m) -> n i m", i=5)
                    for tt in range(NTH):
                        dst = w0_d[tt, :, nb, :, :].rearrange("i n m -> n i m")
                        nc.scalar.dma_start(
                            dst, w0v[:, :, tt * MT : (tt + 1) * MT]
                        )
                        dstb = w0b_d[tt, :, nb, :, :].rearrange("i n m -> n i m")
                        nc.scalar.dma_start(
                            dstb, w0v16[:, :, tt * MT : (tt + 1) * MT]
                        )

            # ---------------- Phase C: column stage + theta ----------------
            with (
                tc.tile_pool(name="lc2p", bufs=1) as lc2p,
                tc.tile_pool(name="wp", bufs=3) as wp,
                tc.tile_pool(name="zfp", bufs=2) as zfp,
                tc.tile_pool(name="wbp", bufs=3) as wbp,
                tc.tile_pool(name="evp", bufs=2) as evp,
                tc.tile_pool(name="wps", bufs=3, space="PSUM") as wps,
                tc.tile_pool(name="thps", bufs=1, space="PSUM") as thps,
            ):
                lc2t = []
                for k in range(KB):
                    t_ = lc2p.tile([P, N], F32R, tag=f"lc{k}", name=f"lc2t{k}")
                    nc.sync.dma_start(t_[:], lc2_d[k * P : (k + 1) * P, :])
                    lc2t.append(t_)

                for t in range(NTH):
                    wcur = [[None] * KB for _ in range(5)]
                    # j = 0: load W0 third from DRAM scratch
                    for b in range(KB):
                        w = wp.tile([P, 5 * MT], F32R, tag=f"w{b}", name=f"w0t{b}")
                        src = w0_d[t, :, b, :, :].rearrange("i n m -> n i m")
                        nc.sync.dma_start(w.rearrange("n (i m) -> n i m", i=5), src)
                        wcur[0][b] = w
                    # j = 1: W1 = (Lc2 @ W0) / 2
                    for nb in range(KB):
                        ps = wps.tile([P, 5 * MT], F32, tag="wrec", name="wps1")
                        for k in range(KB):
                            nc.tensor.matmul(
                                ps[:],
                                lhsT=lc2t[k][:, nb * P : (nb + 1) * P],
                                rhs=wcur[0][k][:],
                                start=(k == 0),
                                stop=(k == KB - 1),
                            )
                        w = wp.tile([P, 5 * MT], F32R, tag=f"w{nb}", name=f"w1t{nb}")
                        nc.vector.tensor_scalar_mul(w[:], ps[:], 0.5)
                        wcur[1][nb] = w
                        wb = wbp.tile([P, 5 * MT], BF16, tag="wb", name="wb1")
                        nc.vector.tensor_scalar_mul(wb[:], ps[:], 0.5)
                        nc.gpsimd.dma_start(
                            w_d[t, 0, :, nb, :, :].rearrange("i n m -> n i m"),
                            wb.rearrange("n (i m) -> n i m", i=5),
                        )
                    # j = 2..4: Wj = Lc2 @ W{j-1} - W{j-2}
                    for j in range(2, 5):
                        for nb in range(KB):
                            ps = wps.tile([P, 5 * MT], F32, tag="wrec", name="wpsj")
                            for k in range(KB):
                                nc.tensor.matmul(
                                    ps[:],
                                    lhsT=lc2t[k][:, nb * P : (nb + 1) * P],
                                    rhs=wcur[j - 1][k][:],
                                    start=(k == 0),
                                    stop=False,
                                )
                            nc.tensor.matmul(
                                ps[:],
                                lhsT=negit[:],
                                rhs=wcur[j - 2][nb][:],
                                start=False,
                                stop=True,
                            )
                            w = wp.tile(
                                [P, 5 * MT], F32R, tag=f"w{nb}", name=f"w{j}t{nb}"
                            )
                            nc.vector.tensor_copy(w[:], ps[:])
                            wcur[j][nb] = w
                            wb = wbp.tile([P, 5 * MT], BF16, tag="wb", name=f"wb{j}")
                            nc.vector.tensor_copy(wb[:], ps[:])
                            nc.gpsimd.dma_start(
                                w_d[t, j - 1, :, nb, :, :].rearrange("i n m -> n i m"),
                                wb.rearrange("n (i m) -> n i m", i=5),
                            )

                    # theta contraction per n-block b:
                    #   Zf[(j*5+i), n*MT + m] = W_j[b*128+n, i*MT+m]  (from DRAM)
                    #   psum[kk][(c,o), (n128, m4)]; ev[(c,o), (m16, n128)]
                    for b in range(KB):
                        zf = zfp.tile([25, P * MT], BF16, tag="zf", name="zf")
                        dstv = zf.rearrange("p (n m) -> p n m", n=P)
                        engs = [nc.sync, nc.scalar, nc.gpsimd]
                        engs[b % 3].dma_start(dstv[0:5], w0b_d[t, :, b, :, :])
                        for j in range(1, 5):
                            engs[(b + j) % 3].dma_start(
                                dstv[j * 5 : (j + 1) * 5], w_d[t, j - 1, :, b, :, :]
                            )
                        pss = [
                            thps.tile([P, 512], F32, tag=f"th{kk}", name=f"thps{kk}")
                            for kk in range(4)
                        ]
                        for kk in range(4):
                            for c in range(4):
                                m0 = c * 16 + kk * 4
                                nc.tensor.matmul(
                                    pss[kk][c * 32 : (c + 1) * 32, :],
                                    lhsT=thsb[:],
                                    rhs=dstv[:, :, m0 : m0 + 4],
                                    start=True,
                                    stop=True,
                                    tile_position=(0, c * 32),
                                )
                        ev = evp.tile([P, 16 * P], F32, tag="ev", name="ev")
                        evv = ev.rearrange("p (ml n) -> p ml n", n=P)
                        for kk in range(4):
                            dst = evv[:, kk * 4 : (kk + 1) * 4, :]
                            srcp = pss[kk].rearrange("p (n m) -> p m n", m=4)
                            nc.vector.tensor_scalar_add(dst, srcp, biast[:])
                        for c in range(4):
                            dst = out_d[
                                :,
                                t * MT + c * 16 : t * MT + (c + 1) * 16,
                                b * P : (b + 1) * P,
                            ]
                            srcc = ev[c * 32 : (c + 1) * 32, :].rearrange(
                                "o (ml n) -> o ml n", n=P
                            )
                            nc.scalar.dma_start(dst, srcc)

    nc.finalize()
    return nc


def _host_inputs(x, Lr, Lc, theta, bias):
    x2 = np.ascontiguousarray(x.reshape(M, N), dtype=np.float32)
    lr2 = np.ascontiguousarray(2.0 * Lr, dtype=np.float32)
    lc2 = np.ascontiguousarray(2.0 * Lc, dtype=np.float32)
    thf = np.zeros((25, OUT), np.float32)
    th = theta.reshape(5, 5, OUT)
    for i in range(5):
        for j in range(5):
            thf[j * 5 + i] = th[i, j]
    thf = thf.astype(ml_dtypes.bfloat16)
    biasr = np.ascontiguousarray(
        np.tile(bias.astype(np.float32), 4).reshape(P, 1)
    )
    negi = np.ascontiguousarray(-np.eye(P, dtype=np.float32))
    maps = []
    for c in range(NCORES):
        b01 = np.zeros((M, 2 * BP), np.float32)
        b01[c * ML : (c + 1) * ML, 0:ML] = np.eye(ML, dtype=np.float32)
        b01[:, BP : BP + ML] = Lr[:, c * ML : (c + 1) * ML]
        maps.append(
            {
                "x": x2,
                "lr2": lr2,
                "lc2": lc2,
                "b01": b01,
                "negi": negi,
                "thetaf": thf,
                "biasr": biasr,
            }
        )
    return maps


def kernel(x, Lr, Lc, theta, bias):
    global _BUILT
    from concourse.bass_utils import run_bass_kernel_spmd

    if _BUILT is None:
        _BUILT = _build_program()
    nc = _BUILT
    in_maps = _host_inputs(
        np.asarray(x), np.asarray(Lr), np.asarray(Lc), np.asarray(theta), np.asarray(bias)
    )
    res = run_bass_kernel_spmd(nc, in_maps, core_ids=list(range(NCORES)))
    out = np.concatenate(
        [res.results[c]["outc"] for c in range(NCORES)], axis=1
    )
    return np.ascontiguousarray(out, dtype=np.float32)



# revision 2
# speedup vs baseline: 1.6169x; 1.6169x over previous
"""Trainium2 Bass kernel for nn_BilinearChebConv (bilinear Chebyshev graph conv).

out[o] = sum_{i,j} theta[i,j,0,o] * T_i(Lr) @ x @ T_j(Lc) + bias[o]

Structure (per core c, rows m_c = c*192 .. c*192+191):
    B_i = T_i(Lr)[:, m_c]            (thin column-slice Chebyshev recursion)
    W_0 = x^T @ [B_0 .. B_4]         (n on partitions, (i, m) stacked on free)
    W_j = 2 Lc W_{j-1} - W_{j-2}     (all 5 i-chains in one recursion)
    out[o, m_c, n] = sum_ij theta[ijo] W_j[n, (i, m)] + bias[o]

v2 changes vs baseline:
  - bf16 everywhere on the heavy path (psum accumulation stays fp32);
    x / W0 tiles fit resident in SBUF, so no fp32 W0 DRAM round-trip.
  - Spill of W_j to DRAM uses the SBUF-native (n, (i,m)) element order so
    the write side is 640B-contiguous; the (25, n*m) gather for the theta
    stage reads it with 128B runs (one DMA for j=1..4, one for j=0).
  - theta matmuls read the gathered tile through an m-major strided view so
    PSUM lands in (m, n) order; output staged per third as a (128=(c,o),
    16m x 1536n) bf16 tile and written with 4 fat DMAs (49KB/partition).
  - Output tensor is bf16; host converts to fp32.
"""

import sys

sys.path.insert(0, "/opt/trn_rl_repo")

import numpy as np
import ml_dtypes

M = 1536
N = 1536
NCORES = 8
ML = M // NCORES          # 192 rows per core
MT = 64                   # m-third width (192 = 3 * 64)
NTH = ML // MT            # 3 thirds
OUT = 32
KB = M // 128             # 12 partition blocks
P = 128
BP = ML                   # per-i column block in B (192, no padding in bf16)

_BUILT = None


def _build_program():
    import concourse.bacc as bacc
    import concourse.mybir as mybir
    from concourse import tile

    F32 = mybir.dt.float32
    BF16 = mybir.dt.bfloat16

    nc = bacc.Bacc(num_devices=NCORES)

    x_d = nc.dram_tensor("x", [M, N], BF16, kind="ExternalInput")
    lr2_d = nc.dram_tensor("lr2", [M, M], BF16, kind="ExternalInput")
    lc2_d = nc.dram_tensor("lc2", [N, N], BF16, kind="ExternalInput")
    # b01: cols 0..191 = E_c (identity slice), cols 192..383 = Lr[:, m_c]
    b01_d = nc.dram_tensor("b01", [M, 2 * BP], BF16, kind="ExternalInput")
    negi_d = nc.dram_tensor("negi", [P, P], BF16, kind="ExternalInput")
    thf_d = nc.dram_tensor("thetaf", [25, OUT], BF16, kind="ExternalInput")
    bias_d = nc.dram_tensor("biasr", [P, 1], F32, kind="ExternalInput")
    # W0 spill, full width, SBUF-native order: [nb][n][(i, m192)]
    wd0_d = nc.dram_tensor("wd0", [KB, P, 5 * ML], BF16, kind="Internal")
    # W_{1..4} spill per third: [t][nb][j-1][i][n][m64] — (j,i) adjacent so
    # the theta gather is a single 3D-AP DMA with 128B runs on the write side
    wd_d = nc.dram_tensor("wd", [NTH, KB, 4, 5, P, MT], BF16, kind="Internal")
    out_d = nc.dram_tensor("outc", [OUT, ML, N], BF16, kind="ExternalOutput")

    with tile.TileContext(nc) as tc:
        with tc.tile_pool(name="const", bufs=1) as constp:
            negit = constp.tile([P, P], BF16, tag="negi")
            nc.sync.dma_start(negit[:], negi_d[:])
            thsb = constp.tile([25, OUT], BF16, tag="thf")
            nc.sync.dma_start(thsb[:], thf_d[:])
            biast = constp.tile([P, 1], F32, tag="bias")
            nc.sync.dma_start(biast[:], bias_d[:])

            # w0res persists from Phase R into Phase C
            with tc.tile_pool(name="w0res", bufs=1) as w0p:
                w0res = []
                for nb in range(KB):
                    w0res.append(
                        w0p.tile([P, 5 * ML], BF16, tag=f"w0_{nb}", name=f"w0res{nb}")
                    )

                # ---------------- Phase R: row stage ----------------
                with (
                    tc.tile_pool(name="lr2p", bufs=1) as lr2p,
                    tc.tile_pool(name="bpad", bufs=1) as bpadp,
                    tc.tile_pool(name="xs", bufs=1) as xp,
                    tc.tile_pool(name="brps", bufs=2, space="PSUM") as brps,
                    tc.tile_pool(name="w0ps", bufs=2, space="PSUM") as w0ps,
                ):
                    bt = []
                    for k in range(KB):
                        t_ = bpadp.tile([P, 5 * BP], BF16, tag=f"bp{k}", name=f"bt{k}")
                        [nc.scalar, nc.sync][k % 2].dma_start(
                            t_[:, 0 : 2 * BP], b01_d[k * P : (k + 1) * P, :]
                        )
                        bt.append(t_)
                    xt = []
                    for k in range(KB):
                        t_ = xp.tile([P, N], BF16, tag=f"x{k}", name=f"xt{k}")
                        nc.gpsimd.dma_start(t_[:], x_d[k * P : (k + 1) * P, :])
                        xt.append(t_)
                    lr2t = []
                    for k in range(KB):
                        t_ = lr2p.tile([P, M], BF16, tag=f"lr{k}", name=f"lr2t{k}")
                        [nc.sync, nc.scalar][k % 2].dma_start(
                            t_[:], lr2_d[k * P : (k + 1) * P, :]
                        )
                        lr2t.append(t_)

                    import concourse.mybir as mybir
                    SUB0 = mybir.AluOpType.subtract
                    MULT0 = mybir.AluOpType.mult

                    # W0 segment A: i = 0, 1 (cols 0:384) — only needs x + b01,
                    # so it runs while lr2 loads / B-rec warm up
                    for nb in range(KB):
                        psA = w0ps.tile([P, 2 * BP], F32, tag="w0sA", name="w0psA")
                        for k in range(KB):
                            nc.tensor.matmul(
                                psA[:],
                                lhsT=xt[k][:, nb * P : (nb + 1) * P],
                                rhs=bt[k][:, 0 : 2 * BP],
                                start=(k == 0),
                                stop=(k == KB - 1),
                            )
                        nc.vector.tensor_copy(w0res[nb][:, 0 : 2 * BP], psA[:])

                    # B recursion: B_i = Lr2 @ B_{i-1} - B_{i-2}, i = 2..4
                    for i in range(2, 5):
                        for p in range(KB):
                            ps = brps.tile([P, BP], F32, tag="brec", name="brps")
                            for k in range(KB):
                                nc.tensor.matmul(
                                    ps[:],
                                    lhsT=lr2t[k][:, p * P : (p + 1) * P],
                                    rhs=bt[k][:, (i - 1) * BP : i * BP],
                                    start=(k == 0),
                                    stop=(k == KB - 1),
                                )
                            nc.vector.scalar_tensor_tensor(
                                bt[p][:, i * BP : (i + 1) * BP],
                                ps[:],
                                1.0,
                                bt[p][:, (i - 2) * BP : (i - 1) * BP],
                                MULT0,
                                SUB0,
                            )

                    # W0 segment B: i = 2..4 (cols 384:960), after B-rec
                    segs = [(2 * BP, 512), (2 * BP + 512, 3 * BP - 512)]
                    for nb in range(KB):
                        pss = [
                            w0ps.tile([P, sz], F32, tag=f"w0s{si}", name=f"w0ps{si}")
                            for si, (_, sz) in enumerate(segs)
                        ]
                        for k in range(KB):
                            for ps, (off, sz) in zip(pss, segs):
                                nc.tensor.matmul(
                                    ps[:],
                                    lhsT=xt[k][:, nb * P : (nb + 1) * P],
                                    rhs=bt[k][:, off : off + sz],
                                    start=(k == 0),
                                    stop=(k == KB - 1),
                                )
                        for ps, (off, sz) in zip(pss, segs):
                            nc.vector.tensor_copy(w0res[nb][:, off : off + sz], ps[:])
                        w0v3 = w0res[nb].rearrange("n (i m) -> n i m", i=5)
                        for si in range(NST):
                            [nc.sync, nc.scalar, nc.gpsimd][(nb + si) % 3].dma_start(
                                wd_s[si][nb, 0].rearrange("i n m -> n i m"),
                                w0v3[:, :, OFFS[si] : OFFS[si] + MTS[si]],
                            )

                # ---------------- Phase C: column stage + theta ----------------
                with (
                    tc.tile_pool(name="lc2p", bufs=1) as lc2p,
                    tc.tile_pool(name="wp", bufs=4) as wp,
                    tc.tile_pool(name="zfp", bufs=2) as zfp,
                    tc.tile_pool(name="evp", bufs=1) as evp,
                    tc.tile_pool(name="wps", bufs=3, space="PSUM") as wps,
                    tc.tile_pool(name="thps", bufs=1, space="PSUM") as thps,
                ):
                    lc2t = []
                    for k in range(KB):
                        t_ = lc2p.tile([P, N], BF16, tag=f"lc{k}", name=f"lc2t{k}")
                        [nc.sync, nc.scalar, nc.gpsimd][k % 3].dma_start(
                            t_[:], lc2_d[k * P : (k + 1) * P, :]
                        )
                        lc2t.append(t_)

                    def jrec_chunks(t):
                        """Yield j-recursion chunks (one per (j, nb)) for third t."""
                        wcur = [[None] * KB for _ in range(5)]
                        w0v = [
                            w0res[k]
                            .rearrange("n (i m) -> n i m", i=5)[
                                :, :, t * MT : (t + 1) * MT
                            ]
                            for k in range(KB)
                        ]

                        for j in range(1, 5):
                            for nb in range(KB):
                                def chunk(j=j, nb=nb):
                                    ps = wps.tile(
                                        [P, 5 * MT], F32, tag="wrec", name=f"wps{j}"
                                    )
                                    rhs_prev = (
                                        w0v if j == 1 else [w[:] for w in wcur[j - 1]]
                                    )
                                    for k in range(KB):
                                        nc.tensor.matmul(
                                            ps[:],
                                            lhsT=lc2t[k][:, nb * P : (nb + 1) * P],
                                            rhs=rhs_prev[k],
                                            start=(k == 0),
                                            stop=(k == KB - 1) if j == 1 else False,
                                        )
                                    if j >= 2:
                                        rhs_pp = (
                                            w0v[nb]
                                            if j == 2
                                            else wcur[j - 2][nb][:]
                                        )
                                        nc.tensor.matmul(
                                            ps[:],
                                            lhsT=negit[:],
                                            rhs=rhs_pp,
                                            start=False,
                                            stop=True,
                                        )
                                    w = wp.tile(
                                        [P, 5 * MT], BF16, tag=f"w_{nb}",
                                        name=f"w{j}t{nb}",
                                    )
                                    if j == 1:
                                        nc.vector.tensor_scalar_mul(w[:], ps[:], 0.5)
                                    else:
                                        nc.vector.tensor_copy(w[:], ps[:])
                                    wcur[j][nb] = w
                                    eng = nc.sync if nb % 2 == 0 else nc.scalar
                                    eng.dma_start(
                                        wd_d[t, nb, j - 1].rearrange("i n m -> n i m"),
                                        w[:],
                                    )
                                yield chunk

                    def theta_pair(t, bp, ev):
                        """theta for b-blocks 2*bp, 2*bp+1 off one gathered tile."""
                        evv = ev.rearrange("p (ml n) -> p ml n", n=N)
                        zf = zfp.tile([25, 2 * P * MT], BF16, tag="zf", name="zf")
                        zv = zf.rearrange("p (b n m) -> p b n m", b=2, n=P)
                        engs = [nc.gpsimd, nc.sync, nc.scalar]
                        for h in range(2):
                            b = 2 * bp + h
                            src0 = wd0_d[b].rearrange("n (i m) -> i n m", i=5)[
                                :, :, t * MT : (t + 1) * MT
                            ]
                            engs[(2 * bp + h) % 3].dma_start(zv[0:5, h], src0)
                            src14 = wd_d[t, b].rearrange("j i n m -> (j i) n m")
                            engs[(2 * bp + h + 1) % 3].dma_start(zv[5:25, h], src14)
                        zm = zf.rearrange("p (b n m) -> p b m n", b=2, n=P)
                        for h in range(2):
                            b = 2 * bp + h
                            pss = [
                                thps.tile(
                                    [P, 512], F32, tag=f"th{kk}", name=f"thps{kk}"
                                )
                                for kk in range(4)
                            ]
                            for kk in range(4):
                                for c in range(4):
                                    m0 = c * 16 + kk * 4
                                    nc.tensor.matmul(
                                        pss[kk][c * 32 : (c + 1) * 32, :],
                                        lhsT=thsb[:],
                                        rhs=zm[:, h, m0 : m0 + 4, :],
                                        start=True,
                                        stop=True,
                                        tile_position=(0, c * 32),
                                    )
                            for kk in range(4):
                                dst = evv[
                                    :, kk * 4 : (kk + 1) * 4, b * P : (b + 1) * P
                                ]
                                srcp = pss[kk].rearrange("p (m n) -> p m n", m=4)
                                nc.vector.tensor_scalar_add(dst, srcp, biast[:])

                    def out_dmas(t, ev, third, sixth=False):
                        if sixth:
                            n0, n1 = third * (N // 6), (third + 1) * (N // 6)
                        else:
                            n0, n1 = third * (N // 3), (third + 1) * (N // 3)
                        for c in range(4):
                            dst = out_d[
                                :, t * MT + c * 16 : t * MT + (c + 1) * 16, n0:n1
                            ]
                            srcc = ev[c * 32 : (c + 1) * 32, :].rearrange(
                                "o (ml n) -> o ml n", n=N
                            )[:, :, n0:n1]
                            [nc.scalar, nc.sync, nc.gpsimd][c % 3].dma_start(dst, srcc)

                    # software pipeline: theta(t) interleaves with j-rec(t+1)
                    for chunk in jrec_chunks(0):
                        chunk()
                    for t in range(NTH):
                        ev = evp.tile([P, 16 * N], BF16, tag="ev", name=f"ev{t}")
                        nxt = list(jrec_chunks(t + 1)) if t + 1 < NTH else []
                        done = 0
                        for ci, chunk in enumerate(nxt):
                            chunk()
                            # one theta b-pair after every 8th j-rec chunk
                            if ci % 8 == 7 and done < KB // 2:
                                theta_pair(t, done, ev)
                                done += 1
                                if done in (2, 4):
                                    out_dmas(t, ev, done // 2 - 1)
                        while done < KB // 2:
                            theta_pair(t, done, ev)
                            done += 1
                            if done in (2, 4):
                                out_dmas(t, ev, done // 2 - 1)
                        out_dmas(t, ev, 2)

    nc.finalize()
    return nc


def _host_inputs(x, Lr, Lc, theta, bias):
    bf = ml_dtypes.bfloat16
    x2 = np.ascontiguousarray(x.reshape(M, N)).astype(bf)
    lr2 = np.ascontiguousarray(2.0 * Lr).astype(bf)
    lc2 = np.ascontiguousarray(2.0 * Lc).astype(bf)
    thf = np.zeros((25, OUT), np.float32)
    th = theta.reshape(5, 5, OUT)
    for i in range(5):
        for j in range(5):
            thf[j * 5 + i] = th[i, j]
    thf = thf.astype(bf)
    biasr = np.ascontiguousarray(
        np.tile(bias.astype(np.float32), 4).reshape(P, 1)
    )
    negi = np.ascontiguousarray(-np.eye(P, dtype=np.float32)).astype(bf)
    maps = []
    for c in range(NCORES):
        b01 = np.zeros((M, 2 * BP), np.float32)
        b01[c * ML : (c + 1) * ML, 0:ML] = np.eye(ML, dtype=np.float32)
        b01[:, BP : BP + ML] = Lr[:, c * ML : (c + 1) * ML]
        maps.append(
            {
                "x": x2,
                "lr2": lr2,
                "lc2": lc2,
                "b01": b01.astype(bf),
                "negi": negi,
                "thetaf": thf,
                "biasr": biasr,
            }
        )
    return maps


_RUNNER = None


def _make_runner(nc):
    """Build a cached jitted SPMD executor for the program (the stock
    run_bass_kernel_spmd re-traces and re-jits on every call, which costs
    seconds of host time per launch; this path jits once)."""
    import jax
    import numpy as _np
    import concourse.mybir as mybir
    from concourse import bass2jax as b2j
    from jax.experimental.shard_map import shard_map
    from jax.sharding import Mesh, PartitionSpec

    b2j.install_neuronx_cc_hook()

    partition_name = nc.partition_id_tensor.name if nc.partition_id_tensor else None
    in_names, out_names, out_avals, zero_outs = [], [], [], []
    for alloc in nc.m.functions[0].allocations:
        if not isinstance(alloc, mybir.MemoryLocationSet):
            continue
        name = alloc.memorylocations[0].name
        if alloc.kind == "ExternalInput":
            if name != partition_name:
                in_names.append(name)
        elif alloc.kind == "ExternalOutput":
            shape = tuple(alloc.tensor_shape)
            dtype = mybir.dt.np(alloc.dtype)
            out_names.append(name)
            out_avals.append(jax.core.ShapedArray(shape, dtype))
            zero_outs.append(_np.zeros(shape, dtype))
    n_params = len(in_names)
    all_names = list(in_names) + list(out_names)
    if partition_name is not None:
        all_names.append(partition_name)
    donate = tuple(range(n_params, n_params + len(out_names)))

    def _body(*args):
        operands = list(args)
        if partition_name is not None:
            operands.append(b2j.partition_id_tensor())
        return tuple(
            b2j._bass_exec_p.bind(
                *operands,
                out_avals=tuple(out_avals),
                in_names=tuple(all_names),
                out_names=tuple(out_names),
                lowering_input_output_aliases=(),
                sim_require_finite=True,
                sim_require_nnan=True,
                nc=nc,
            )
        )

    devices = jax.devices()[:NCORES]
    mesh = Mesh(_np.asarray(devices), ("core",))
    nio = n_params + len(out_names)
    sharded = jax.jit(
        shard_map(
            _body,
            mesh=mesh,
            in_specs=(PartitionSpec("core"),) * nio,
            out_specs=(PartitionSpec("core"),) * len(out_names),
            check_rep=False,
        ),
        donate_argnums=donate,
        keep_unused=True,
    )

    def run(in_maps):
        concat_in = [
            _np.concatenate([m[name] for m in in_maps], axis=0)
            for name in in_names
        ]
        concat_zeros = [
            _np.zeros((NCORES * z.shape[0], *z.shape[1:]), z.dtype)
            for z in zero_outs
        ]
        out_arrs = sharded(*concat_in, *concat_zeros)
        return {
            name: _np.asarray(out_arrs[i]).reshape(
                NCORES, *out_avals[i].shape
            )
            for i, name in enumerate(out_names)
        }

    return run


def kernel(x, Lr, Lc, theta, bias):
    global _BUILT, _RUNNER
    if _BUILT is None:
        _BUILT = _build_program()
    if _RUNNER is None:
        _RUNNER = _make_runner(_BUILT)
    in_maps = _host_inputs(
        np.asarray(x), np.asarray(Lr), np.asarray(Lc), np.asarray(theta), np.asarray(bias)
    )
    res = _RUNNER(in_maps)
    out = np.concatenate(
        [np.asarray(res["outc"][c], dtype=np.float32) for c in range(NCORES)],
        axis=1,
    )
    return np.ascontiguousarray(out)


# revision 4
# speedup vs baseline: 2.1928x; 1.3562x over previous
"""Trainium2 Bass kernel for nn_BilinearChebConv (bilinear Chebyshev graph conv).

out[o] = sum_{i,j} theta[i,j,0,o] * T_i(Lr) @ x @ T_j(Lc) + bias[o]

Structure (per core c, rows m_c = c*192 .. c*192+191):
    B_i = T_i(Lr)[:, m_c]            (thin column-slice Chebyshev recursion)
    W_0 = x^T @ [B_0 .. B_4]         (n on partitions, (i, m) stacked on free)
    W_j = 2 Lc W_{j-1} - W_{j-2}     (all 5 i-chains in one recursion)
    out[o, m_c, n] = sum_ij theta[ijo] W_j[n, (i, m)] + bias[o]

v2 changes vs baseline:
  - bf16 everywhere on the heavy path (psum accumulation stays fp32);
    x / W0 tiles fit resident in SBUF, so no fp32 W0 DRAM round-trip.
  - Spill of W_j to DRAM uses the SBUF-native (n, (i,m)) element order so
    the write side is 640B-contiguous; the (25, n*m) gather for the theta
    stage reads it with 128B runs (one DMA for j=1..4, one for j=0).
  - theta matmuls read the gathered tile through an m-major strided view so
    PSUM lands in (m, n) order; output staged per third as a (128=(c,o),
    16m x 1536n) bf16 tile and written with 4 fat DMAs (49KB/partition).
  - Output tensor is bf16; host converts to fp32.
"""

import sys

sys.path.insert(0, "/opt/trn_rl_repo")

import numpy as np
import ml_dtypes

M = 1536
N = 1536
NCORES = 8
ML = M // NCORES          # 192 rows per core
MT = 64                   # m-third width (192 = 3 * 64)
NTH = ML // MT            # 3 thirds
OUT = 32
KB = M // 128             # 12 partition blocks
P = 128
BP = ML                   # per-i column block in B (192, no padding in bf16)

_BUILT = None


def _build_program():
    import concourse.bacc as bacc
    import concourse.mybir as mybir
    from concourse import tile

    F32 = mybir.dt.float32
    BF16 = mybir.dt.bfloat16

    nc = bacc.Bacc(num_devices=NCORES)

    x_d = nc.dram_tensor("x", [M, N], BF16, kind="ExternalInput")
    lr2_d = nc.dram_tensor("lr2", [M, M], BF16, kind="ExternalInput")
    lc2_d = nc.dram_tensor("lc2", [N, N], BF16, kind="ExternalInput")
    # b01: cols 0..191 = E_c (identity slice), cols 192..383 = Lr[:, m_c]
    b01_d = nc.dram_tensor("b01", [M, 2 * BP], BF16, kind="ExternalInput")
    negi_d = nc.dram_tensor("negi", [P, P], BF16, kind="ExternalInput")
    thf_d = nc.dram_tensor("thetaf", [25, OUT], BF16, kind="ExternalInput")
    bias_d = nc.dram_tensor("biasr", [P, 1], F32, kind="ExternalInput")
    # W0 spill, full width, SBUF-native order: [nb][n][(i, m192)]
    wd0_d = nc.dram_tensor("wd0", [KB, P, 5 * ML], BF16, kind="Internal")
    # W_{1..4} spill per third: [t][nb][j-1][i][n][m64] — (j,i) adjacent so
    # the theta gather is a single 3D-AP DMA with 128B runs on the write side
    wd_d = nc.dram_tensor("wd", [NTH, KB, 4, 5, P, MT], BF16, kind="Internal")
    out_d = nc.dram_tensor("outc", [OUT, ML, N], BF16, kind="ExternalOutput")

    with tile.TileContext(nc) as tc:
        with tc.tile_pool(name="const", bufs=1) as constp:
            negit = constp.tile([P, P], BF16, tag="negi")
            nc.sync.dma_start(negit[:], negi_d[:])
            thsb = constp.tile([25, OUT], BF16, tag="thf")
            nc.sync.dma_start(thsb[:], thf_d[:])
            biast = constp.tile([P, 1], F32, tag="bias")
            nc.sync.dma_start(biast[:], bias_d[:])

            # w0res persists from Phase R into Phase C
            with tc.tile_pool(name="w0res", bufs=1) as w0p:
                w0res = []
                for nb in range(KB):
                    w0res.append(
                        w0p.tile([P, 5 * ML], BF16, tag=f"w0_{nb}", name=f"w0res{nb}")
                    )

                # ---------------- Phase R: row stage ----------------
                with (
                    tc.tile_pool(name="lr2p", bufs=1) as lr2p,
                    tc.tile_pool(name="bpad", bufs=1) as bpadp,
                    tc.tile_pool(name="xs", bufs=1) as xp,
                    tc.tile_pool(name="brps", bufs=2, space="PSUM") as brps,
                    tc.tile_pool(name="w0ps", bufs=2, space="PSUM") as w0ps,
                ):
                    bt = []
                    for k in range(KB):
                        t_ = bpadp.tile([P, 5 * BP], BF16, tag=f"bp{k}", name=f"bt{k}")
                        [nc.scalar, nc.sync][k % 2].dma_start(
                            t_[:, 0 : 2 * BP], b01_d[k * P : (k + 1) * P, :]
                        )
                        bt.append(t_)
                    xt = []
                    for kp in range(KB // 2):
                        t_ = xp.tile([P, 2 * N], BF16, tag=f"x{kp}", name=f"xt{kp}")
                        dst = t_.rearrange("p (k n) -> p k n", k=2)
                        src = x_d[2 * kp * P : (2 * kp + 2) * P, :].rearrange(
                            "(k p) n -> p k n", k=2
                        )
                        nc.gpsimd.dma_start(dst, src)
                        xt.append(t_[:, 0:N])
                        xt.append(t_[:, N : 2 * N])
                    lr2t = []
                    for kp in range(KB // 2):
                        t_ = lr2p.tile([P, 2 * M], BF16, tag=f"lr{kp}", name=f"lr2t{kp}")
                        dst = t_.rearrange("p (k n) -> p k n", k=2)
                        src = lr2_d[2 * kp * P : (2 * kp + 2) * P, :].rearrange(
                            "(k p) n -> p k n", k=2
                        )
                        [nc.sync, nc.scalar][kp % 2].dma_start(dst, src)
                        lr2t.append(t_[:, 0:M])
                        lr2t.append(t_[:, M : 2 * M])

                    import concourse.mybir as mybir
                    SUB0 = mybir.AluOpType.subtract
                    MULT0 = mybir.AluOpType.mult

                    # W0 segment A: i = 0, 1 (cols 0:384) — only needs x + b01,
                    # so it runs while lr2 loads / B-rec warm up
                    for nb in range(KB):
                        psA = w0ps.tile([P, 2 * BP], F32, tag="w0sA", name="w0psA")
                        for k in range(KB):
                            nc.tensor.matmul(
                                psA[:],
                                lhsT=xt[k][:, nb * P : (nb + 1) * P],
                                rhs=bt[k][:, 0 : 2 * BP],
                                start=(k == 0),
                                stop=(k == KB - 1),
                            )
                        nc.vector.tensor_copy(w0res[nb][:, 0 : 2 * BP], psA[:])

                    # B recursion: B_i = Lr2 @ B_{i-1} - B_{i-2}, i = 2..4
                    for i in range(2, 5):
                        for p in range(KB):
                            ps = brps.tile([P, BP], F32, tag="brec", name="brps")
                            for k in range(KB):
                                nc.tensor.matmul(
                                    ps[:],
                                    lhsT=lr2t[k][:, p * P : (p + 1) * P],
                                    rhs=bt[k][:, (i - 1) * BP : i * BP],
                                    start=(k == 0),
                                    stop=(k == KB - 1),
                                )
                            nc.vector.scalar_tensor_tensor(
                                bt[p][:, i * BP : (i + 1) * BP],
                                ps[:],
                                1.0,
                                bt[p][:, (i - 2) * BP : (i - 1) * BP],
                                MULT0,
                                SUB0,
                            )

                    # W0 segment B: i = 2..4 (cols 384:960), after B-rec
                    segs = [(2 * BP, 512), (2 * BP + 512, 3 * BP - 512)]
                    for nb in range(KB):
                        pss = [
                            w0ps.tile([P, sz], F32, tag=f"w0s{si}", name=f"w0ps{si}")
                            for si, (_, sz) in enumerate(segs)
                        ]
                        for k in range(KB):
                            for ps, (off, sz) in zip(pss, segs):
                                nc.tensor.matmul(
                                    ps[:],
                                    lhsT=xt[k][:, nb * P : (nb + 1) * P],
                                    rhs=bt[k][:, off : off + sz],
                                    start=(k == 0),
                                    stop=(k == KB - 1),
                                )
                        for ps, (off, sz) in zip(pss, segs):
                            nc.vector.tensor_copy(w0res[nb][:, off : off + sz], ps[:])
                        w0v3 = w0res[nb].rearrange("n (i m) -> n i m", i=5)
                        for si in range(NST):
                            [nc.sync, nc.scalar, nc.gpsimd][(nb + si) % 3].dma_start(
                                wd_s[si][nb, 0].rearrange("i n m -> n i m"),
                                w0v3[:, :, OFFS[si] : OFFS[si] + MTS[si]],
                            )

                # ---------------- Phase C: column stage + theta ----------------
                with (
                    tc.tile_pool(name="lc2p", bufs=1) as lc2p,
                    tc.tile_pool(name="wp", bufs=4) as wp,
                    tc.tile_pool(name="zfp", bufs=2) as zfp,
                    tc.tile_pool(name="evp", bufs=1) as evp,
                    tc.tile_pool(name="wps", bufs=2, space="PSUM") as wps,
                    tc.tile_pool(name="thps", bufs=2, space="PSUM") as thps,
                ):
                    lc2t = []
                    for k in range(KB):
                        t_ = lc2p.tile([P, N], BF16, tag=f"lc{k}", name=f"lc2t{k}")
                        [nc.sync, nc.scalar, nc.gpsimd][k % 3].dma_start(
                            t_[:], lc2_d[k * P : (k + 1) * P, :]
                        )
                        lc2t.append(t_)

                    def jrec_chunks(t):
                        """Yield j-recursion chunks (one per (j, nb)) for third t."""
                        wcur = [[None] * KB for _ in range(5)]
                        w0v = [
                            w0res[k]
                            .rearrange("n (i m) -> n i m", i=5)[
                                :, :, t * MT : (t + 1) * MT
                            ]
                            for k in range(KB)
                        ]

                        for j in range(1, 5):
                            for nb in range(KB):
                                def chunk(j=j, nb=nb):
                                    ps = wps.tile(
                                        [P, 5 * MT], F32, tag="wrec", name=f"wps{j}"
                                    )
                                    rhs_prev = (
                                        w0v if j == 1 else [w[:] for w in wcur[j - 1]]
                                    )
                                    for k in range(KB):
                                        nc.tensor.matmul(
                                            ps[:],
                                            lhsT=lc2t[k][:, nb * P : (nb + 1) * P],
                                            rhs=rhs_prev[k],
                                            start=(k == 0),
                                            stop=(k == KB - 1) if j == 1 else False,
                                        )
                                    if j >= 2:
                                        rhs_pp = (
                                            w0v[nb]
                                            if j == 2
                                            else wcur[j - 2][nb][:]
                                        )
                                        nc.tensor.matmul(
                                            ps[:],
                                            lhsT=negit[:],
                                            rhs=rhs_pp,
                                            start=False,
                                            stop=True,
                                        )
                                    w = wp.tile(
                                        [P, 5 * MT], BF16, tag=f"w_{nb}",
                                        name=f"w{j}t{nb}",
                                    )
                                    if j == 1:
                                        nc.vector.tensor_scalar_mul(w[:], ps[:], 0.5)
                                    else:
                                        nc.vector.tensor_copy(w[:], ps[:])
                                    wcur[j][nb] = w
                                    eng = nc.sync if nb % 2 == 0 else nc.scalar
                                    eng.dma_start(
                                        wd_d[t, nb, j - 1].rearrange("i n m -> n i m"),
                                        w[:],
                                    )
                                yield chunk

                    def theta_pair(t, bp, ev):
                        """theta for b-blocks 2*bp, 2*bp+1 off one gathered tile."""
                        evv = ev.rearrange("p (ml n) -> p ml n", n=N)
                        zf = zfp.tile([25, 2 * P * MT], BF16, tag="zf", name="zf")
                        zv = zf.rearrange("p (b n m) -> p b n m", b=2, n=P)
                        engs = [nc.gpsimd, nc.sync, nc.scalar]
                        for h in range(2):
                            b = 2 * bp + h
                            src0 = wd0_d[b].rearrange("n (i m) -> i n m", i=5)[
                                :, :, t * MT : (t + 1) * MT
                            ]
                            engs[(2 * bp + h) % 3].dma_start(zv[0:5, h], src0)
                            src14 = wd_d[t, b].rearrange("j i n m -> (j i) n m")
                            engs[(2 * bp + h + 1) % 3].dma_start(zv[5:25, h], src14)
                        zm = zf.rearrange("p (b n m) -> p b m n", b=2, n=P)
                        for h in range(2):
                            b = 2 * bp + h
                            pss = [
                                thps.tile(
                                    [P, 512], F32, tag=f"th{kk}", name=f"thps{kk}"
                                )
                                for kk in range(4)
                            ]
                            for kk in range(4):
                                for c in range(4):
                                    m0 = c * 16 + kk * 4
                                    nc.tensor.matmul(
                                        pss[kk][c * 32 : (c + 1) * 32, :],
                                        lhsT=thsb[:],
                                        rhs=zm[:, h, m0 : m0 + 4, :],
                                        start=True,
                                        stop=True,
                                        tile_position=(0, c * 32),
                                    )
                            for kk in range(4):
                                dst = evv[
                                    :, kk * 4 : (kk + 1) * 4, b * P : (b + 1) * P
                                ]
                                srcp = pss[kk].rearrange("p (m n) -> p m n", m=4)
                                nc.vector.tensor_scalar_add(dst, srcp, biast[:])

                    def out_dmas(t, ev, third, sixth=False):
                        if sixth:
                            n0, n1 = third * (N // 6), (third + 1) * (N // 6)
                        else:
                            n0, n1 = third * (N // 3), (third + 1) * (N // 3)
                        for c in range(4):
                            dst = out_d[
                                :, t * MT + c * 16 : t * MT + (c + 1) * 16, n0:n1
                            ]
                            srcc = ev[c * 32 : (c + 1) * 32, :].rearrange(
                                "o (ml n) -> o ml n", n=N
                            )[:, :, n0:n1]
                            [nc.scalar, nc.sync, nc.gpsimd][c % 3].dma_start(dst, srcc)

                    # software pipeline: theta(t) interleaves with j-rec(t+1)
                    for chunk in jrec_chunks(0):
                        chunk()
                    for t in range(NTH):
                        ev = evp.tile([P, 16 * N], BF16, tag="ev", name=f"ev{t}")
                        nxt = list(jrec_chunks(t + 1)) if t + 1 < NTH else []
                        done = 0
                        for ci, chunk in enumerate(nxt):
                            chunk()
                            # one theta b-pair after every 8th j-rec chunk
                            if ci % 8 == 7 and done < KB // 2:
                                theta_pair(t, done, ev)
                                done += 1
                                if done in (2, 4):
                                    out_dmas(t, ev, done // 2 - 1)
                        while done < KB // 2:
                            theta_pair(t, done, ev)
                            done += 1
                            if done in (2, 4):
                                out_dmas(t, ev, done // 2 - 1)
                        out_dmas(t, ev, 2)

    nc.finalize()
    return nc


def _host_inputs(x, Lr, Lc, theta, bias):
    bf = ml_dtypes.bfloat16
    x2 = np.ascontiguousarray(x.reshape(M, N)).astype(bf)
    lr2 = np.ascontiguousarray(2.0 * Lr).astype(bf)
    lc2 = np.ascontiguousarray(2.0 * Lc).astype(bf)
    thf = np.zeros((25, OUT), np.float32)
    th = theta.reshape(5, 5, OUT)
    for i in range(5):
        for j in range(5):
            thf[j * 5 + i] = th[i, j]
    thf = thf.astype(bf)
    biasr = np.ascontiguousarray(
        np.tile(bias.astype(np.float32), 4).reshape(P, 1)
    )
    negi = np.ascontiguousarray(-np.eye(P, dtype=np.float32)).astype(bf)
    maps = []
    for c in range(NCORES):
        b01 = np.zeros((M, 2 * BP), np.float32)
        b01[c * ML : (c + 1) * ML, 0:ML] = np.eye(ML, dtype=np.float32)
        b01[:, BP : BP + ML] = Lr[:, c * ML : (c + 1) * ML]
        maps.append(
            {
                "x": x2,
                "lr2": lr2,
                "lc2": lc2,
                "b01": b01.astype(bf),
                "negi": negi,
                "thetaf": thf,
                "biasr": biasr,
            }
        )
    return maps


_RUNNER = None


def _make_runner(nc):
    """Build a cached jitted SPMD executor for the program (the stock
    run_bass_kernel_spmd re-traces and re-jits on every call, which costs
    seconds of host time per launch; this path jits once)."""
    import jax
    import numpy as _np
    import concourse.mybir as mybir
    from concourse import bass2jax as b2j
    from jax.experimental.shard_map import shard_map
    from jax.sharding import Mesh, PartitionSpec

    b2j.install_neuronx_cc_hook()

    partition_name = nc.partition_id_tensor.name if nc.partition_id_tensor else None
    in_names, out_names, out_avals, zero_outs = [], [], [], []
    for alloc in nc.m.functions[0].allocations:
        if not isinstance(alloc, mybir.MemoryLocationSet):
            continue
        name = alloc.memorylocations[0].name
        if alloc.kind == "ExternalInput":
            if name != partition_name:
                in_names.append(name)
        elif alloc.kind == "ExternalOutput":
            shape = tuple(alloc.tensor_shape)
            dtype = mybir.dt.np(alloc.dtype)
            out_names.append(name)
            out_avals.append(jax.core.ShapedArray(shape, dtype))
            zero_outs.append(_np.zeros(shape, dtype))
    n_params = len(in_names)
    all_names = list(in_names) + list(out_names)
    if partition_name is not None:
        all_names.append(partition_name)
    donate = tuple(range(n_params, n_params + len(out_names)))

    def _body(*args):
        operands = list(args)
        if partition_name is not None:
            operands.append(b2j.partition_id_tensor())
        return tuple(
            b2j._bass_exec_p.bind(
                *operands,
                out_avals=tuple(out_avals),
                in_names=tuple(all_names),
                out_names=tuple(out_names),
                lowering_input_output_aliases=(),
                sim_require_finite=True,
                sim_require_nnan=True,
                nc=nc,
            )
        )

    devices = jax.devices()[:NCORES]
    mesh = Mesh(_np.asarray(devices), ("core",))
    nio = n_params + len(out_names)
    sharded = jax.jit(
        shard_map(
            _body,
            mesh=mesh,
            in_specs=(PartitionSpec("core"),) * nio,
            out_specs=(PartitionSpec("core"),) * len(out_names),
            check_rep=False,
        ),
        donate_argnums=donate,
        keep_unused=True,
    )

    def run(in_maps):
        concat_in = [
            _np.concatenate([m[name] for m in in_maps], axis=0)
            for name in in_names
        ]
        concat_zeros = [
            _np.zeros((NCORES * z.shape[0], *z.shape[1:]), z.dtype)
            for z in zero_outs
        ]
        out_arrs = sharded(*concat_in, *concat_zeros)
        return {
            name: _np.asarray(out_arrs[i]).reshape(
                NCORES, *out_avals[i].shape
            )
            for i, name in enumerate(out_names)
        }

    return run


def kernel(x, Lr, Lc, theta, bias):
    global _BUILT, _RUNNER
    if _BUILT is None:
        _BUILT = _build_program()
    if _RUNNER is None:
        _RUNNER = _make_runner(_BUILT)
    in_maps = _host_inputs(
        np.asarray(x), np.asarray(Lr), np.asarray(Lc), np.asarray(theta), np.asarray(bias)
    )
    res = _RUNNER(in_maps)
    out = np.concatenate(
        [np.asarray(res["outc"][c], dtype=np.float32) for c in range(NCORES)],
        axis=1,
    )
    return np.ascontiguousarray(out)
